# revision 13
# baseline (speedup 1.0000x reference)
"""Maxwell rheological model kernel for Trainium2 (8 NeuronCores, SPMD).

Recurrence per batch row (a = E/ETA = 2, E_INFTY = 1, E = 2):
    gamma[0] = 0
    gamma[n+1] = (1 - 2*dt[n]) * gamma[n] + 2*dt[n] * eps[n]
    sigma[n+1] = 3*eps[n+1] - 2*gamma[n+1];  sigma[0] = 0

Mapped onto the DVE TensorTensorScan instruction with g = 2*gamma:
    c[n] = 1 - 2*dt[n]          (ACT engine)
    d[n] = 4*dt[n]*eps[n]       (DVE scalar_tensor_tensor)
    g[n] = c[n]*g[n-1] + d[n]   (DVE tensor_tensor_scan, init 0)
    sigma[:, 1:] = 3*eps[:, 1:] - g[:, :-1]

Batch is sharded across 8 cores (data parallel, no collectives).
"""

import sys

if "/opt/trn_rl_repo" not in sys.path:
    sys.path.insert(0, "/opt/trn_rl_repo")

import numpy as np

import concourse.bacc as bacc
import concourse.mybir as mybir
from concourse.bass_utils import run_bass_kernel_spmd
from concourse.tile import TileContext

B, T = 16384, 2048
N_CORES = 8
B_CORE = B // N_CORES
P = 128
N_STRIPS = B_CORE // P

_prog = None


def _build():
    f32 = mybir.dt.float32
    Alu = mybir.AluOpType
    nc = bacc.Bacc(
        "TRN2",
        target_bir_lowering=False,
        debug=False,
        enable_asserts=False,
    )
    strains = nc.dram_tensor("strains", [B_CORE, T], f32, kind="ExternalInput").ap()
    dts = nc.dram_tensor("dts", [B_CORE, T], f32, kind="ExternalInput").ap()
    out = nc.dram_tensor("out", [B_CORE, T], f32, kind="ExternalOutput").ap()
    with TileContext(nc) as tc:
        with (
            tc.tile_pool(name="pin", bufs=5) as pin,
            tc.tile_pool(name="pc", bufs=4) as pc,
            tc.tile_pool(name="pmid", bufs=3) as pmid,
            tc.tile_pool(name="pout", bufs=4) as pout,
        ):
            for i in range(N_STRIPS):
                r0 = i * P
                # First/last strips are processed in column segments
                # (chained scans) to shorten the pipeline head and tail.
                if i == 0:
                    bounds = [0, 512, 1024, T]
                elif i == N_STRIPS - 1:
                    bounds = [0, 1024, 1536, T]
                else:
                    bounds = [0, T]
                dt_t = pin.tile([P, T], f32, tag="dt")
                ep_t = pin.tile([P, T], f32, tag="eps")
                c_t = pc.tile([P, T - 1], f32, tag="c")
                d_t = pmid.tile([P, T - 1], f32, tag="d")
                g_t = pmid.tile([P, T - 1], f32, tag="g")
                s_t = pout.tile([P, T], f32, tag="sig")
                segs = list(zip(bounds[:-1], bounds[1:]))
                for lo, hi in segs:
                    nc.sync.dma_start(
                        out=dt_t[:, lo:hi], in_=dts[r0 : r0 + P, lo:hi]
                    )
                    nc.sync.dma_start(
                        out=ep_t[:, lo:hi], in_=strains[r0 : r0 + P, lo:hi]
                    )
                # sig[:,0] = 0
                nc.scalar.activation(
                    out=s_t[:, 0:1],
                    in_=dt_t[:, 0:1],
                    func=mybir.ActivationFunctionType.Copy,
                    scale=0.0,
                )
                for lo, hi in segs:
                    ch = min(hi, T - 1)
                    # ACT: c = 1 - 2*dt over [lo, ch)
                    nc.scalar.activation(
                        out=c_t[:, lo:ch],
                        in_=dt_t[:, lo:ch],
                        func=mybir.ActivationFunctionType.Copy,
                        scale=-2.0,
                        bias=1.0,
                    )
                    # DVE: d = (dt*4)*eps over [lo, ch)
                    nc.vector.scalar_tensor_tensor(
                        out=d_t[:, lo:ch],
                        in0=dt_t[:, lo:ch],
                        scalar=4.0,
                        in1=ep_t[:, lo:ch],
                        op0=Alu.mult,
                        op1=Alu.mult,
                    )
                    # DVE: g = scan(c, d) over [lo, ch), carry chained
                    nc.vector.tensor_tensor_scan(
                        out=g_t[:, lo:ch],
                        data0=c_t[:, lo:ch],
                        data1=d_t[:, lo:ch],
                        initial=0.0 if lo == 0 else g_t[:, lo - 1 : lo],
                        op0=Alu.mult,
                        op1=Alu.add,
                    )
                    # DVE: sig[:, m] = 3*eps[:, m] - g[:, m-1] over [max(lo,1), hi)
                    slo = max(lo, 1)
                    nc.vector.scalar_tensor_tensor(
                        out=s_t[:, slo:hi],
                        in0=ep_t[:, slo:hi],
                        scalar=3.0,
                        in1=g_t[:, slo - 1 : hi - 1],
                        op0=Alu.mult,
                        op1=Alu.subtract,
                    )
                    # Store issued from ACT's HWDGE ring: keeps Sync purely
                    # for loads so store waits don't block load issues.
                    nc.scalar.dma_start(
                        out=out[r0 : r0 + P, lo:hi], in_=s_t[:, lo:hi]
                    )
    nc.compile()
    return nc


def _get_prog():
    global _prog
    if _prog is None:
        _prog = _build()
    return _prog


def _run(strains, dts, **kwargs):
    nc = _get_prog()
    ss = np.split(np.ascontiguousarray(strains, dtype=np.float32), N_CORES, axis=0)
    ds = np.split(np.ascontiguousarray(dts, dtype=np.float32), N_CORES, axis=0)
    in_maps = [{"strains": s, "dts": d} for s, d in zip(ss, ds)]
    res = run_bass_kernel_spmd(nc, in_maps, core_ids=list(range(N_CORES)), **kwargs)
    full = np.concatenate([r["out"] for r in res.results], axis=0)
    return full, res


def kernel(strains, dts):
    out, _ = _run(strains, dts)
    return out


if __name__ == "__main__":
    rng = np.random.default_rng(0)
    eps = rng.standard_normal((B, T), dtype=np.float32)
    dts = rng.random((B, T), dtype=np.float32)
    out = kernel(eps, dts)
    print("ran ok", out.shape, out.dtype)
